# revision 103
# baseline (speedup 1.0000x reference)
"""Self-contained Trainium2 (Bass/Tile) kernel for the AttentionGRUCell
problem: 8-core data-parallel over batch.

Fast path: the big attention matmul (Wa_bot.T @ tanh(es)) and the Va
contraction run in fp8e4 DoubleRow mode (2 k-tiles per pass); the
ct-dependent GRU weight matmuls run in bf16 with all tail weights
prefetched into SBUF during the attention loop, so the GRU tail is
compute-dense.  End-to-end rel err ~2e-3 (gate 2e-2).

kernel(**inputs) takes the FULL unsharded inputs and returns the FULL
[512, 1088] output ([alpha, h_new] per row), running the Bass program on
NeuronCores 0-7 via run_bass_kernel_spmd.
"""
import sys

for _p in ("/opt/trn_rl_repo",):
    if _p not in sys.path:
        sys.path.insert(0, _p)

import numpy as np
import concourse.bass as bass
import concourse.mybir as mybir
import concourse.tile as tile
import bass_rust
from concourse.alu_op_type import AluOpType
from concourse.masks import make_identity
from concourse.vector_clock import ScopedClock

F32 = mybir.dt.float32
F32R = mybir.dt.float32r
BF16 = mybir.dt.bfloat16
F8 = mybir.dt.float8e4
AF = mybir.ActivationFunctionType
AX = mybir.AxisListType
DR = mybir.MatmulPerfMode.DoubleRow

Bc, T, XD, ED, U = 64, 64, 512, 1024, 1024
NSBLK = 8
N_CORES = 8
B_FULL = 512
WA_SCALE = 64.0
VA_SCALE = 32.0


# ---------------------------------------------------------------------------
# Workarounds for this walrus build: instructions may carry at most one sem
# wait ("Too many sync wait commands"), including the Tile kernel-tail drain.
# ---------------------------------------------------------------------------

def _patched_drain_and_barrier(self, tick_clock, wait_clock):
    nc = self.nc
    probe = nc.sync.nop(nofuse=True)
    wait_clock.add_sem_waits(probe.ins, ScopedClock({None: tick_clock.global_clock}))
    si = probe.ins.sync_info
    waits = list(si.on_wait) if si is not None else []
    probe.ins.sync_info = bass_rust.SyncInfo(on_wait=waits[:1], on_update=[])
    for w in waits[1:]:
        n2 = nc.sync.nop(nofuse=True)
        n2.ins.sync_info = bass_rust.SyncInfo(on_wait=[w], on_update=[])
    nc.sync.drain()
    nc.all_engine_barrier()
    assert self.sems is not None
    popped = nc._tile_sem_poison_stack.pop()
    assert popped is self._sem_poison
    nc.clear_and_free_semaphores(list(self.sems.allocated().values()))
    nc.all_engine_barrier()


tile.TileContext._drain_and_barrier = _patched_drain_and_barrier

_fix_ctr = [0]


def fix_multi_waits(nc, max_waits=1):
    """Hoist extra sem waits onto same-engine InstNoOps placed immediately
    before the instruction -- engines execute in order, so semantics are
    identical."""
    for f in nc.m.functions:
        for blk in f.blocks:
            insts = blk.instructions
            if not any(
                i.sync_info is not None and len(i.sync_info.on_wait) > max_waits
                for i in insts
            ):
                continue
            out = []
            for inst in insts:
                si = inst.sync_info
                if si is not None and len(si.on_wait) > max_waits:
                    waits = list(si.on_wait)
                    for w in waits[:-max_waits]:
                        _fix_ctr[0] += 1
                        nop = mybir.InstNoOp(
                            name=f"waitfix-{_fix_ctr[0]}",
                            ins=[],
                            outs=[],
                            engine=inst.engine,
                        )
                        nop.sync_info = bass_rust.SyncInfo(on_wait=[w], on_update=[])
                        out.append(nop)
                    inst.sync_info = bass_rust.SyncInfo(
                        on_wait=waits[-max_waits:], on_update=list(si.on_update)
                    )
                out.append(inst)
            blk.instructions = out


# ---------------------------------------------------------------------------
# Kernel program
# ---------------------------------------------------------------------------

def build_nc():
    nc = bass.Bass("TRN2", target_bir_lowering=False, debug=False)

    inputs_d = nc.dram_tensor("inputs", [Bc, XD], F32, kind="ExternalInput")
    h_d = nc.dram_tensor("h", [Bc, U], F32, kind="ExternalInput")
    es_d = nc.dram_tensor("encoder_states", [Bc, T, ED], F32R, kind="ExternalInput")
    kernel_d = nc.dram_tensor("kernel", [XD + ED, 3 * U], F32R, kind="ExternalInput")
    rk_d = nc.dram_tensor("recurrent_kernel", [U, 3 * U], F32R, kind="ExternalInput")
    bias_d = nc.dram_tensor("bias", [3 * U], F32, kind="ExternalInput")
    wa_d = nc.dram_tensor("Wa", [U + ED, U], F32R, kind="ExternalInput")
    va_d = nc.dram_tensor("Va", [U, 1], F32, kind="ExternalInput")
    out_d = nc.dram_tensor("out", [Bc, T + U], F32, kind="ExternalOutput")

    es_flat = es_d.ap().rearrange("b t e -> (b t) e")

    with tile.TileContext(nc) as tc:
        with (
            tc.tile_pool(name="singles", bufs=1) as sg,
            tc.tile_pool(name="esr", bufs=4) as esr_pool,
            tc.tile_pool(name="tes8", bufs=8) as tes8_pool,
            tc.tile_pool(name="gt8", bufs=4) as gt8_pool,
            tc.tile_pool(name="wk", bufs=8) as wk_pool,
            tc.tile_pool(name="stg", bufs=4) as stg_pool,
            tc.tile_pool(name="stt", bufs=2) as stt_pool,
            tc.tile_pool(name="smalls", bufs=4) as sm_pool,
            # one shared PSUM scope, 8 banks total (the GRU tail reuses the
            # attention tags for its six gate accumulators):
            tc.tile_pool(name="ps_tr", bufs=2, space="PSUM") as ps_tr,
            tc.tile_pool(name="ps_v", bufs=2, space="PSUM") as ps_v,
            tc.tile_pool(name="ps_acc", bufs=2, space="PSUM") as ps_acc,
            tc.tile_pool(name="ps_e", bufs=1, space="PSUM") as ps_e,
            tc.tile_pool(name="ps_ct", bufs=1, space="PSUM") as ps_ct,
        ):
            # ---- earliest loads: h (starts the th chain), Wa_top (qk path),
            # Wa_bot, es ----
            h_sb = sg.tile([Bc, U], F32)
            nc.sync.dma_start(out=h_sb[:], in_=h_d[:])

            # one tile per es superblock, parallel-queue DMA triggers on the
            # gpsimd queue set (kept clear of the bulk weight streams, which
            # run on sync).  Per-queue DMA bandwidth is ~20 GB/s.
            def load_esr(g, nsplit=2):
                e_t = esr_pool.tile([128, 4, ED], F32R, tag="esr", name=f"esr{g}")
                W = ED // nsplit
                for rr in range(4):
                    t_idx = 4 * g + rr
                    for hf in range(nsplit):
                        nc.sync.dma_start(
                            out=e_t[:, rr, W * hf:W * (hf + 1)],
                            in_=es_flat[
                                128 * t_idx:128 * (t_idx + 1),
                                W * hf:W * (hf + 1),
                            ],
                        )
                return e_t

            # Wa_bot (es half): load fp32, cast to fp8 with x64 scale into the
            # DoubleRow pair layout [128, pair, sub, U].  Triggers go on the
            # scalar+gpsimd queues to keep sync free for es/h.
            wab8 = sg.tile([128, 4, 2, U], F8)
            wab_stage = []
            for j in range(8):
                st = stg_pool.tile([128, U], F32R, tag="stg", name=f"wast{j}")
                eng = nc.scalar if j % 2 == 0 else nc.gpsimd
                for hf in range(2):
                    eng.dma_start(
                        out=st[:, 512 * hf:512 * (hf + 1)],
                        in_=wa_d.ap()[
                            U + 128 * j:U + 128 * (j + 1),
                            512 * hf:512 * (hf + 1),
                        ],
                    )
                wab_stage.append(st)

            esr = load_esr(0)

            # Wa_top tiles for qk on the gpsimd queue (empty at startup, and
            # keeps sync free so es0/es1 land early); qk matmuls then fill
            # the PE while es0 is still in flight
            wat_tiles = []
            for dd in range(8):
                wat = stg_pool.tile([128, U], F32R, tag="stg", name=f"wat{dd}")
                for hf in range(2):
                    nc.gpsimd.dma_start(
                        out=wat[:, 512 * hf:512 * (hf + 1)],
                        in_=wa_d.ap()[
                            128 * dd:128 * (dd + 1), 512 * hf:512 * (hf + 1)
                        ],
                    )
                wat_tiles.append(wat)

            in_sb = sg.tile([Bc, XD], F32, tag="scr_in_r")
            nc.sync.dma_start(out=in_sb[:], in_=inputs_d[:])

            ident = sg.tile([128, 128], F32)
            make_identity(nc, ident[:])
            identR = sg.tile([128, 128], F32R)
            nc.vector.tensor_copy(identR[:], ident[:])

            for j in range(8):
                nc.vector.tensor_scalar_mul(
                    wab8[:, j // 2, j % 2, :], wab_stage[j][:], WA_SCALE
                )

            # ---- th + small transposes (PE work with no weight deps) ----
            th = sg.tile([Bc, U], F32, tag="scr_th_z")
            nc.scalar.activation(out=th[:], in_=h_sb[:], func=AF.Tanh)

            _tp_ctr = [0]

            def transpose_to(dst, src_2d, j):
                # src_2d: [Bc, 128] -> dst[:, j, :] = src.T
                _tp_ctr[0] += 1
                pt = ps_tr.tile([128, Bc], F32, tag="tr", name=f"tp{_tp_ctr[0]}")
                nc.tensor.transpose(pt[:], src_2d, ident[:Bc, :Bc])
                nc.vector.tensor_copy(dst[:, j, :], pt[:])

            thT = sg.tile([128, 8, Bc], F32R, tag="scr_thT_ctT")
            hT = sg.tile([128, 8, Bc], F32R, tag="scr_hT_rhT")
            inT = sg.tile([128, 4, Bc], F32R, tag="scr_inT_hh")
            for j in range(8):
                transpose_to(thT, th[:, 128 * j:128 * (j + 1)], j)
            for j in range(8):
                transpose_to(hT, h_sb[:, 128 * j:128 * (j + 1)], j)
            for j in range(4):
                transpose_to(inT, in_sb[:, 128 * j:128 * (j + 1)], j)

            # qk = th @ Wa_top computed in natural layout (N=512 matmuls),
            # then transposed into qkT [u on partitions, b free]
            va_sb = sg.tile([128, 8], F32)
            nc.sync.dma_start(
                out=va_sb[:], in_=va_d.ap().rearrange("(j p) o -> p (j o)", p=128)
            )
            # dual-fp8 ldweights needs >=32 weight columns: pad with zeros,
            # only psum row 0 is meaningful
            va8 = sg.tile([128, 8, 32], F8)
            nc.vector.memset(va8[:].rearrange("p j o -> p (j o)"), 0.0)
            nc.vector.tensor_scalar_mul(
                va8[:, :, 0:1].rearrange("p j o -> p (j o)"), va_sb[:], VA_SCALE
            )
            qkT = sg.tile([128, 8, Bc], F32)
            qk_nat = sg.tile([Bc, U], F32, tag="scr_qk_ct")
            pqs = [
                ps_acc.tile([Bc, 512], F32, tag="acc", name=f"pq{h}")
                for h in range(2)
            ]
            for d in range(8):
                for half in range(2):
                    nc.tensor.matmul(
                        pqs[half][:],
                        thT[:, d, :],
                        wat_tiles[d][:, 512 * half:512 * (half + 1)],
                        start=(d == 0),
                        stop=(d == 7),
                    )
            for half in range(2):
                nc.vector.tensor_copy(
                    qk_nat[:, 512 * half:512 * (half + 1)], pqs[half][:]
                )
            for j in range(8):
                transpose_to(qkT, qk_nat[:, 128 * j:128 * (j + 1)], j)

            nxt = load_esr(1)

            bias_b = sg.tile([Bc, 3 * U], F32)
            bias_src = bias_d.ap()
            nc.sync.dma_start(
                out=bias_b[:],
                in_=bass.AP(
                    tensor=bias_src.tensor,
                    offset=bias_src.offset,
                    ap=[[0, Bc], list(bias_src.ap[0])],
                ),
            )

            # masks for block-diag A build
            masks = sg.tile([128, 4, 8], F32)
            nc.vector.memset(masks[:], 0.0)
            for rr in range(4):
                nc.vector.memset(masks[0:64, rr, 2 * rr:2 * rr + 1], 1.0)
                nc.vector.memset(masks[64:128, rr, 2 * rr + 1:2 * rr + 2], 1.0)

            ct_sb = sg.tile([Bc, ED], F32, tag="scr_qk_ct")
            gh_sb = sg.tile([Bc, 2 * U], F32, tag="scr_gh_t1")
            gx0_sb = sg.tile([Bc, 3 * U], F32)

            # GRU ct-part z/r weights, prefetched in fp8 DoubleRow pair
            # layout during the loop (x64 scale; the ct operand carries x32,
            # undone by 1/2048 in the gate STT).  hh columns stream at the
            # tail.  kb16[:, d//2, d%2, :] = kernel[512+128d:., 0:2U] * 64
            kb16 = sg.tile([128, 4, 2, 2 * U], F8)

            def kb16_block(i):
                # load order: column block (z, r) outer, d inner
                part, d = divmod(i, 8)
                st = stg_pool.tile([128, U], F32R, tag="stg", name=f"kbs{i}")
                for hf in range(2):
                    nc.scalar.dma_start(
                        out=st[:, 512 * hf:512 * (hf + 1)],
                        in_=kernel_d.ap()[
                            512 + 128 * d:512 + 128 * (d + 1),
                            U * part + 512 * hf:U * part + 512 * (hf + 1),
                        ],
                    )
                nc.vector.tensor_scalar_mul(
                    kb16[:, d // 2, d % 2, U * part:U * (part + 1)],
                    st[:], WA_SCALE,
                )

            # --- spread weight-stream blocks (emitted inside the sblk loop) ---
            def gh_block(n):
                # gh[:, n] = h @ rk[:, n-slice] + bias[n-slice]
                # issue all 8 DMAs first so the transfers run in parallel
                pg = ps_acc.tile([Bc, 512], F32, tag="acc", name=f"pg{n}")
                rkts = []
                for d in range(8):
                    rkt = wk_pool.tile(
                        [128, 512], F32R, tag="wk", name=f"rkt{n}_{d}"
                    )
                    nc.sync.dma_start(
                        out=rkt[:],
                        in_=rk_d.ap()[
                            128 * d:128 * (d + 1), 512 * n:512 * (n + 1)
                        ],
                    )
                    rkts.append(rkt)
                for d in range(8):
                    nc.tensor.matmul(
                        pg[:], hT[:, d, :], rkts[d][:],
                        start=(d == 0), stop=(d == 7),
                    )
                nc.vector.scalar_tensor_tensor(
                    out=gh_sb[:, 512 * n:512 * (n + 1)],
                    in0=pg[:],
                    scalar=1.0,
                    in1=bias_b[:, 512 * n:512 * (n + 1)],
                    op0=AluOpType.mult,
                    op1=AluOpType.add,
                )

            def gx0_block(n):
                # inputs-part of the x @ kernel gates
                pa = ps_acc.tile([Bc, 512], F32, tag="acc", name=f"gx0_{n}")
                wkts = []
                for d in range(4):
                    wkt = wk_pool.tile(
                        [128, 512], F32R, tag="wk", name=f"wk0_{n}_{d}"
                    )
                    nc.sync.dma_start(
                        out=wkt[:],
                        in_=kernel_d.ap()[
                            128 * d:128 * (d + 1), 512 * n:512 * (n + 1)
                        ],
                    )
                    wkts.append(wkt)
                for d in range(4):
                    nc.tensor.matmul(
                        pa[:], inT[:, d, :], wkts[d][:],
                        start=(d == 0), stop=(d == 3),
                    )
                nc.vector.tensor_copy(gx0_sb[:, 512 * n:512 * (n + 1)], pa[:])

            # --- pipelined alpha -> c_t pieces (ehat = unnormalized exp) ---
            def ct_head(g, ehat):
                pat = ps_ct.tile([64, 8], F32, tag="ct", name=f"pat{g}")
                nc.tensor.transpose(pat[:], ehat[:], ident[:8, :8])
                alpT2 = sm_pool.tile([128, 8], F32, tag="alT2", name=f"aT2{g}")
                nc.vector.tensor_copy(alpT2[0:64, :], pat[:])
                nc.gpsimd.dma_start(out=alpT2[64:128, :], in_=alpT2[0:64, :])
                ars = []
                for rr in range(4):
                    a_r = sm_pool.tile([128, 8], F32R, tag="A", name=f"A{g}_{rr}")
                    nc.gpsimd.tensor_mul(a_r[:], alpT2[:], masks[:, rr, :])
                    ars.append(a_r)
                return ars

            def ct_tail(g, esr_g, ars, srec, gather=True):
                # pct = sum_t ehat_t * es_t ; normalize by 1/sum(ehat) on copy
                ct_stage = sm_pool.tile(
                    [8, ED], F32, tag="ctst", name=f"cts{g}", bufs=1
                )
                for n in range(2):
                    pct = ps_ct.tile([8, 512], F32, tag="ct", name=f"pct{g}_{n}")
                    for rr in range(4):
                        nc.tensor.matmul(
                            pct[:],
                            ars[rr][:],
                            esr_g[:, rr, 512 * n:512 * (n + 1)],
                            start=(rr == 0),
                            stop=(rr == 3),
                        )
                    nc.vector.tensor_scalar_mul(
                        ct_stage[:, 512 * n:512 * (n + 1)], pct[:], srec[:]
                    )
                if gather:
                    nc.gpsimd.dma_start(
                        out=ct_sb[8 * g:8 * (g + 1), :], in_=ct_stage[:]
                    )
                return ct_stage

            # ---- attention superblock loop ----
            prev = None   # (g, esr, ehat, srec) of previous superblock
            kb_emitted = [0]
            for g in range(NSBLK):
                if g + 2 < NSBLK:
                    nxt2 = load_esr(g + 2)
                else:
                    nxt2 = None

                # tes8[p][:, q, :] = fp8(tanh(es).T) for j = 2p+q
                tes8 = []
                for p in range(4):
                    t8 = tes8_pool.tile(
                        [128, 2, 512], F8, tag="tes8", name=f"tes{g}_{p}"
                    )
                    for q in range(2):
                        j = 2 * p + q
                        pt = ps_tr.tile(
                            [128, 512], F32R, tag="tr", name=f"ptr{g}_{j}"
                        )
                        for rr in range(4):
                            nc.tensor.transpose(
                                pt[:, 128 * rr:128 * (rr + 1)],
                                esr[:, rr, 128 * j:128 * (j + 1)],
                                identR[:],
                            )
                        nc.scalar.activation(
                            out=t8[:, q, :], in_=pt[:], func=AF.Tanh
                        )
                    tes8.append(t8)

                # gt8 = fp8(tanh(Wa_bot.T @ tes / WA_SCALE + qk bcast))
                ars_prev = None
                gt8s = []
                for u in range(8):
                    if g == 0:
                        # STT(u) waits on qkT which lands late at startup:
                        # rotate over 4 psum banks (ps_e/ps_ct are idle until
                        # the first ct chain) so the u-loop runs 4 deep
                        pvpool, pvtag = [
                            (ps_v, "v"), (ps_v, "v"), (ps_e, "e"), (ps_ct, "ct")
                        ][u % 4]
                        pv = pvpool.tile(
                            [128, 512], F32, tag=pvtag, name=f"pv{g}_{u}"
                        )
                    else:
                        pv = ps_v.tile([128, 512], F32, tag="v", name=f"pv{g}_{u}")
                    for p in range(4):
                        nc.tensor.matmul(
                            pv[:],
                            wab8[:, p, :, 128 * u:128 * (u + 1)],
                            tes8[p][:],
                            start=(p == 0),
                            stop=(p == 3),
                            perf_mode=DR,
                        )
                    if u % 2 == 0:
                        g8 = gt8_pool.tile(
                            [128, 2, 512], F8, tag="gt8", name=f"gt{g}_{u // 2}"
                        )
                        gt8s.append(g8)
                    qk_slice = qkT[:, u, 8 * g:8 * g + 8]
                    qk_bc = bass.AP(
                        tensor=qk_slice.tensor,
                        offset=qk_slice.offset,
                        ap=[
                            list(qk_slice.ap[0]),
                            list(qk_slice.ap[1]),
                            [0, T],
                        ],
                    )
                    sc = stt_pool.tile(
                        [128, 512], F32, tag="stt", name=f"sc{g}_{u}"
                    )
                    nc.vector.scalar_tensor_tensor(
                        out=sc[:],
                        in0=pv[:],
                        scalar=1.0 / WA_SCALE,
                        in1=qk_bc,
                        op0=AluOpType.mult,
                        op1=AluOpType.add,
                    )
                    nc.scalar.activation(
                        out=gt8s[u // 2][:, u % 2, :], in_=sc[:], func=AF.Tanh
                    )
                    if u == 3 and prev is not None:
                        ars_prev = ct_head(prev[0], prev[2])
                    if u == 6 and prev is not None:
                        ct_tail(prev[0], prev[1], ars_prev, prev[3])

                # e*VA_SCALE = Va8.T @ gt8  (row 0 of [32, 512])
                pe = ps_e.tile([32, 512], F32, tag="e", name=f"pe{g}")
                for p in range(4):
                    nc.tensor.matmul(
                        pe[:],
                        va8[:, 2 * p:2 * p + 2, :],
                        gt8s[p][:],
                        start=(p == 0),
                        stop=(p == 3),
                        perf_mode=DR,
                    )

                # spread GRU weight streams across the attention phase
                if 4 <= g <= 7:
                    gh_block(g - 4)
                if 2 <= g <= 7:
                    gx0_block(g - 2)

                # softmax over t (|e| <~ 1.5: exp w/o max-sub is safe in fp32)
                e_sb = sm_pool.tile(
                    [1, 512], F32, tag="ctst", name=f"esb{g}", bufs=1
                )
                nc.vector.tensor_copy(e_sb[:], pe[0:1, :])
                ehat = sm_pool.tile([8, T], F32, tag="al", name=f"al{g}", bufs=2)
                nc.gpsimd.dma_start(
                    out=ehat[:],
                    in_=e_sb[0:1, :].rearrange("p (b t) -> p b t", b=8),
                )
                nc.scalar.activation(
                    out=ehat[:], in_=ehat[:], func=AF.Exp, scale=1.0 / VA_SCALE
                )
                ssum = sm_pool.tile([8, 1], F32, tag="ssum", name=f"ss{g}")
                nc.vector.reduce_sum(ssum[:], ehat[:], AX.X)
                srec = sm_pool.tile([8, 1], F32, tag="srec", name=f"sr{g}")
                nc.vector.reciprocal(srec[:], ssum[:])
                alpha = sm_pool.tile([8, T], F32, tag="alo", name=f"alo{g}", bufs=2)
                nc.gpsimd.tensor_scalar_mul(alpha[:], ehat[:], srec[:])
                nc.gpsimd.dma_start(
                    out=out_d.ap()[8 * g:8 * (g + 1), 0:T], in_=alpha[:]
                )

                # kb16 z/r tiles, emitted at the END of the superblock so the
                # scalar triggers and vector casts queue behind this block's
                # latency-critical tanh chain instead of ahead of it
                for _ in range({2: 3, 3: 3, 6: 5, 7: 5}.get(g, 0)):
                    if kb_emitted[0] < 16:
                        kb16_block(kb_emitted[0])
                        kb_emitted[0] += 1

                prev = (g, esr, ehat, srec)
                esr = nxt
                nxt = nxt2

            # fold gx0 into gh (z/r columns) and bias into gx0 (hh columns)
            # while the PE finishes g=7: one STT instead of two in the tail
            for n in range(4):
                nc.vector.scalar_tensor_tensor(
                    out=gh_sb[:, 512 * n:512 * (n + 1)],
                    in0=gh_sb[:, 512 * n:512 * (n + 1)],
                    scalar=1.0,
                    in1=gx0_sb[:, 512 * n:512 * (n + 1)],
                    op0=AluOpType.mult,
                    op1=AluOpType.add,
                )
            for n2 in range(2):
                o = 2 * U + 512 * n2
                nc.vector.scalar_tensor_tensor(
                    out=gx0_sb[:, o:o + 512],
                    in0=gx0_sb[:, o:o + 512],
                    scalar=1.0,
                    in1=bias_b[:, o:o + 512],
                    op0=AluOpType.mult,
                    op1=AluOpType.add,
                )

            # kernel-hh and rk_hh tiles: DMA into freed esr slots on the sync
            # queue (idle at the tail), 16 parallel half-triggers per tile.
            # kbh first: it is consumed earlier and gets the earlier-freed slots
            kbh_stage = []
            for half in range(2):
                ks = esr_pool.tile([128, 4, U], F32R, tag="esr", name=f"kbhs{half}")
                for rr in range(4):
                    dk = 4 * half + rr
                    for hf in range(2):
                        nc.sync.dma_start(
                            out=ks[:, rr, 512 * hf:512 * (hf + 1)],
                            in_=kernel_d.ap()[
                                512 + 128 * dk:512 + 128 * (dk + 1),
                                2 * U + 512 * hf:2 * U + 512 * (hf + 1),
                            ],
                        )
                kbh_stage.append(ks)
            rkh_tiles = []
            for half in range(2):
                rkh = esr_pool.tile([128, 4, U], F32R, tag="esr", name=f"rkh{half}")
                for rr in range(4):
                    dk = 4 * half + rr
                    for hf in range(2):
                        nc.sync.dma_start(
                            out=rkh[:, rr, 512 * hf:512 * (hf + 1)],
                            in_=rk_d.ap()[
                                128 * dk:128 * (dk + 1),
                                2 * U + 512 * hf:2 * U + 512 * (hf + 1),
                            ],
                        )
                rkh_tiles.append(rkh)

            # No-dependency warmup transposes: the PE idles during the g=7
            # softmax chain and drops to a low p-state, making the first
            # ~8µs of tail matmuls ~1.7x slower.  Keep the pipeline warm.
            for w in range(16):
                pw = ps_acc.tile([128, Bc], F32, tag="acc", name=f"warm{w}")
                nc.tensor.transpose(pw[:], ident[:Bc, :], ident[:Bc, :Bc])

            # ctT in fp8 DoubleRow pairs, x32 scale.  Rows 0..55 (g=0..6)
            # have been in ct_sb since g=6: transpose them while the g=7
            # softmax chain runs; rows 56..63 come straight from its
            # ct_stage afterwards (no gather DMA on the critical path).
            ctT = sg.tile([128, 4, 2, Bc], F8, tag="scr_thT_ctT")
            for j in range(8):
                pt = ps_tr.tile([128, 56], F32, tag="tr", name=f"tpa{j}")
                nc.tensor.transpose(
                    pt[:], ct_sb[0:56, 128 * j:128 * (j + 1)], ident[:56, :56]
                )
                nc.vector.tensor_scalar_mul(
                    ctT[:, j // 2, j % 2, 0:56], pt[:], VA_SCALE
                )

            ars_prev = ct_head(prev[0], prev[2])
            cts7 = ct_tail(prev[0], prev[1], ars_prev, prev[3], gather=False)

            for j in range(8):
                pt = ps_tr.tile([128, 8], F32, tag="tr", name=f"tpb{j}")
                nc.tensor.transpose(
                    pt[:], cts7[:, 128 * j:128 * (j + 1)], ident[:8, :8]
                )
                nc.vector.tensor_scalar_mul(
                    ctT[:, j // 2, j % 2, 56:64], pt[:], VA_SCALE
                )

            half_sb = sg.tile([Bc, 1], F32)
            nc.vector.memset(half_sb[:], 0.5)
            z_sb = sg.tile([Bc, U], F32, tag="scr_th_z")
            r_sb = sg.tile([Bc, U], F32, tag="scr_in_r")
            hh_sb = sg.tile([Bc, U], F32, tag="scr_inT_hh")

            # six gate accumulators in the (now idle) attention psum banks
            gx = [
                ps_tr.tile([Bc, 512], F32, tag="tr", name="gxa"),
                ps_tr.tile([Bc, 512], F32, tag="tr", name="gxb"),
                ps_v.tile([Bc, 512], F32, tag="v", name="gxc"),
                ps_v.tile([Bc, 512], F32, tag="v", name="gxd"),
                ps_e.tile([Bc, 512], F32, tag="e", name="gxe"),
                ps_ct.tile([Bc, 512], F32, tag="ct", name="gxf"),
            ]

            GATE_DESCALE = 1.0 / (WA_SCALE * VA_SCALE)

            def add_inplace(pa, n, src_sb, scale=1.0):
                nc.vector.scalar_tensor_tensor(
                    out=pa[:],
                    in0=pa[:],
                    scalar=scale,
                    in1=src_sb[:, 512 * n:512 * (n + 1)],
                    op0=AluOpType.mult,
                    op1=AluOpType.add,
                )

            rhT = sg.tile([128, 8, Bc], F32R, tag="scr_hT_rhT")

            def gate_block(n):
                # hard_sigmoid(gx/2048 + (gh+gx0)) = min(relu(0.2x+0.5), 1)
                dst = z_sb if n < 2 else r_sb
                o = 512 * (n % 2)
                add_inplace(gx[n], n, gh_sb, scale=GATE_DESCALE)
                nc.scalar.activation(
                    out=dst[:, o:o + 512], in_=gx[n][:],
                    func=AF.Relu, bias=half_sb[:], scale=0.2,
                )
                nc.vector.tensor_scalar_min(
                    dst[:, o:o + 512], dst[:, o:o + 512], 1.0
                )
                if n >= 2:
                    # rh = r * h for this half, then its rhT tiles scaled
                    # x2048 so the rh stream matches the fp8 hh psum scale
                    nc.vector.tensor_mul(
                        r_sb[:, o:o + 512], r_sb[:, o:o + 512],
                        h_sb[:, o:o + 512],
                    )
                    for j in range(4 * (n - 2), 4 * (n - 1)):
                        _tp_ctr[0] += 1
                        pt = ps_tr.tile(
                            [128, Bc], F32, tag="tr", name=f"tpr{j}"
                        )
                        nc.tensor.transpose(
                            pt[:], r_sb[:, 128 * j:128 * (j + 1)],
                            ident[:Bc, :Bc],
                        )
                        nc.vector.tensor_scalar_mul(
                            rhT[:, j, :], pt[:], WA_SCALE * VA_SCALE
                        )

            # cast the staged hh ct-part weight tiles to fp8 pairs (x64)
            # via the now-idle wk pool
            kbh_tiles = []
            for pr in range(4):
                kbh = wk_pool.tile([128, 2, U], F8, tag="wk", name=f"kbh{pr}")
                for q in range(2):
                    d = 2 * pr + q
                    nc.vector.tensor_scalar_mul(
                        kbh[:, q, :], kbh_stage[d // 4][:, d % 4, :], WA_SCALE
                    )
                kbh_tiles.append(kbh)

            # ct-part z/r matmuls in fp8 DoubleRow, n-outer; interleave the
            # gate vector math per completed column block
            for n in range(4):
                for p in range(4):
                    nc.tensor.matmul(
                        gx[n][:],
                        ctT[:, p, :, :],
                        kb16[:, p, :, 512 * n:512 * (n + 1)],
                        start=(p == 0),
                        stop=(p == 3),
                        perf_mode=DR,
                    )
                if n == 1:
                    gate_block(0)
                    gate_block(1)
                if n == 3:
                    gate_block(2)
                    gate_block(3)

            # hh ct-part: fp8 DoubleRow over the streamed tail tiles
            for pr in range(4):
                for n2 in range(2):
                    nc.tensor.matmul(
                        gx[4 + n2][:],
                        ctT[:, pr, :, :],
                        kbh_tiles[pr][:, :, 512 * n2:512 * (n2 + 1)],
                        start=(pr == 0),
                        stop=False,
                        perf_mode=DR,
                    )

            # hh accumulators: (r*h) @ rk_hh stream, half-outer so the first
            # half's vector chain overlaps the second half's matmuls
            t1 = sg.tile([Bc, U], F32, tag="scr_gh_t1")

            def hh_chain(n2):
                # hh = tanh(gates_hh/2048 + (gx0+bias)); h_new = hh + z*(h-hh)
                o = 512 * n2
                pa = gx[4 + n2]
                add_inplace(pa, 4 + n2, gx0_sb, scale=GATE_DESCALE)
                nc.scalar.activation(
                    out=hh_sb[:, o:o + 512], in_=pa[:], func=AF.Tanh
                )
                # chunked so the final exposure after the last matmul is short
                for ck in range(2):
                    sl = slice(o + 256 * ck, o + 256 * (ck + 1))
                    nc.vector.tensor_sub(t1[:, sl], h_sb[:, sl], hh_sb[:, sl])
                    nc.vector.tensor_mul(t1[:, sl], z_sb[:, sl], t1[:, sl])
                    nc.vector.tensor_add(t1[:, sl], hh_sb[:, sl], t1[:, sl])
                    nc.sync.dma_start(
                        out=out_d.ap()[:, T + o + 256 * ck:T + o + 256 * (ck + 1)],
                        in_=t1[:, sl],
                    )

            for n2 in range(2):
                for d in range(8):
                    nc.tensor.matmul(
                        gx[4 + n2][:],
                        rhT[:, d, :],
                        rkh_tiles[d // 4][:, d % 4, 512 * n2:512 * (n2 + 1)],
                        start=False,
                        stop=(d == 7),
                    )
                hh_chain(n2)

    return nc

_built = [None]


def kernel(**inputs):
    if _built[0] is None:
        nc = build_nc()
        fix_multi_waits(nc)
        _built[0] = nc
    nc = _built[0]

    from concourse.bass_utils import run_bass_kernel_spmd

    def f32(name):
        return np.ascontiguousarray(np.asarray(inputs[name], dtype=np.float32))

    inp = f32("inputs")
    h = f32("h")
    es = f32("encoder_states")
    ker = f32("kernel")
    rk = f32("recurrent_kernel")
    bias = f32("bias")
    wa = f32("Wa")
    va = f32("Va")

    in_maps = []
    for c in range(N_CORES):
        sl = slice(c * Bc, (c + 1) * Bc)
        in_maps.append({
            "inputs": inp[sl],
            "h": h[sl],
            "encoder_states": es[sl],
            "kernel": ker,
            "recurrent_kernel": rk,
            "bias": bias,
            "Wa": wa,
            "Va": va,
        })

    res = run_bass_kernel_spmd(nc, in_maps, list(range(N_CORES)))
    out = np.concatenate(
        [res.results[c]["out"] for c in range(N_CORES)], axis=0
    ).astype(np.float32)
    return out


# revision 104
# speedup vs baseline: 1.0689x; 1.0689x over previous
"""Self-contained Trainium2 (Bass/Tile) kernel for the AttentionGRUCell
problem: 8-core data-parallel over batch.

Fast path: the big attention matmul (Wa_bot.T @ tanh(es)) and the Va
contraction run in fp8e4 DoubleRow mode (2 k-tiles per pass); the
ct-dependent GRU weight matmuls run in bf16 with all tail weights
prefetched into SBUF during the attention loop, so the GRU tail is
compute-dense.  End-to-end rel err ~2e-3 (gate 2e-2).

kernel(**inputs) takes the FULL unsharded inputs and returns the FULL
[512, 1088] output ([alpha, h_new] per row), running the Bass program on
NeuronCores 0-7 via run_bass_kernel_spmd.
"""
import sys

for _p in ("/opt/trn_rl_repo",):
    if _p not in sys.path:
        sys.path.insert(0, _p)

import numpy as np
import concourse.bass as bass
import concourse.mybir as mybir
import concourse.tile as tile
import bass_rust
from concourse.alu_op_type import AluOpType
from concourse.masks import make_identity
from concourse.vector_clock import ScopedClock

F32 = mybir.dt.float32
F32R = mybir.dt.float32r
BF16 = mybir.dt.bfloat16
F8 = mybir.dt.float8e4
AF = mybir.ActivationFunctionType
AX = mybir.AxisListType
DR = mybir.MatmulPerfMode.DoubleRow

Bc, T, XD, ED, U = 64, 64, 512, 1024, 1024
NSBLK = 8
N_CORES = 8
B_FULL = 512
WA_SCALE = 64.0
VA_SCALE = 32.0


# ---------------------------------------------------------------------------
# Workarounds for this walrus build: instructions may carry at most one sem
# wait ("Too many sync wait commands"), including the Tile kernel-tail drain.
# ---------------------------------------------------------------------------

def _patched_drain_and_barrier(self, tick_clock, wait_clock):
    nc = self.nc
    probe = nc.sync.nop(nofuse=True)
    wait_clock.add_sem_waits(probe.ins, ScopedClock({None: tick_clock.global_clock}))
    si = probe.ins.sync_info
    waits = list(si.on_wait) if si is not None else []
    probe.ins.sync_info = bass_rust.SyncInfo(on_wait=waits[:1], on_update=[])
    for w in waits[1:]:
        n2 = nc.sync.nop(nofuse=True)
        n2.ins.sync_info = bass_rust.SyncInfo(on_wait=[w], on_update=[])
    nc.sync.drain()
    nc.all_engine_barrier()
    assert self.sems is not None
    popped = nc._tile_sem_poison_stack.pop()
    assert popped is self._sem_poison
    nc.clear_and_free_semaphores(list(self.sems.allocated().values()))
    nc.all_engine_barrier()


tile.TileContext._drain_and_barrier = _patched_drain_and_barrier

_fix_ctr = [0]


def fix_multi_waits(nc, max_waits=1):
    """Hoist extra sem waits onto same-engine InstNoOps placed immediately
    before the instruction -- engines execute in order, so semantics are
    identical."""
    for f in nc.m.functions:
        for blk in f.blocks:
            insts = blk.instructions
            if not any(
                i.sync_info is not None and len(i.sync_info.on_wait) > max_waits
                for i in insts
            ):
                continue
            out = []
            for inst in insts:
                si = inst.sync_info
                if si is not None and len(si.on_wait) > max_waits:
                    waits = list(si.on_wait)
                    for w in waits[:-max_waits]:
                        _fix_ctr[0] += 1
                        nop = mybir.InstNoOp(
                            name=f"waitfix-{_fix_ctr[0]}",
                            ins=[],
                            outs=[],
                            engine=inst.engine,
                        )
                        nop.sync_info = bass_rust.SyncInfo(on_wait=[w], on_update=[])
                        out.append(nop)
                    inst.sync_info = bass_rust.SyncInfo(
                        on_wait=waits[-max_waits:], on_update=list(si.on_update)
                    )
                out.append(inst)
            blk.instructions = out


# ---------------------------------------------------------------------------
# Kernel program
# ---------------------------------------------------------------------------

def build_nc():
    nc = bass.Bass("TRN2", target_bir_lowering=False, debug=False)

    inputs_d = nc.dram_tensor("inputs", [Bc, XD], F32, kind="ExternalInput")
    h_d = nc.dram_tensor("h", [Bc, U], F32, kind="ExternalInput")
    es_d = nc.dram_tensor("encoder_states", [Bc, T, ED], F32R, kind="ExternalInput")
    kernel_d = nc.dram_tensor("kernel", [XD + ED, 3 * U], F32R, kind="ExternalInput")
    rk_d = nc.dram_tensor("recurrent_kernel", [U, 3 * U], F32R, kind="ExternalInput")
    bias_d = nc.dram_tensor("bias", [3 * U], F32, kind="ExternalInput")
    wa_d = nc.dram_tensor("Wa", [U + ED, U], F32R, kind="ExternalInput")
    va_d = nc.dram_tensor("Va", [U, 1], F32, kind="ExternalInput")
    out_d = nc.dram_tensor("out", [Bc, T + U], F32, kind="ExternalOutput")

    es_flat = es_d.ap().rearrange("b t e -> (b t) e")

    with tile.TileContext(nc) as tc:
        with (
            tc.tile_pool(name="singles", bufs=1) as sg,
            tc.tile_pool(name="esr", bufs=4) as esr_pool,
            tc.tile_pool(name="tes8", bufs=8) as tes8_pool,
            tc.tile_pool(name="gt8", bufs=4) as gt8_pool,
            tc.tile_pool(name="wk", bufs=8) as wk_pool,
            tc.tile_pool(name="stg", bufs=4) as stg_pool,
            tc.tile_pool(name="stt", bufs=2) as stt_pool,
            tc.tile_pool(name="smalls", bufs=4) as sm_pool,
            # one shared PSUM scope, 8 banks total (the GRU tail reuses the
            # attention tags for its six gate accumulators):
            tc.tile_pool(name="ps_tr", bufs=2, space="PSUM") as ps_tr,
            tc.tile_pool(name="ps_v", bufs=2, space="PSUM") as ps_v,
            tc.tile_pool(name="ps_acc", bufs=2, space="PSUM") as ps_acc,
            tc.tile_pool(name="ps_e", bufs=1, space="PSUM") as ps_e,
            tc.tile_pool(name="ps_ct", bufs=1, space="PSUM") as ps_ct,
        ):
            # ---- earliest loads: h (starts the th chain), Wa_top (qk path),
            # Wa_bot, es ----
            h_sb = sg.tile([Bc, U], F32)
            nc.sync.dma_start(out=h_sb[:], in_=h_d[:])

            # one tile per es superblock, parallel-queue DMA triggers on the
            # gpsimd queue set (kept clear of the bulk weight streams, which
            # run on sync).  Per-queue DMA bandwidth is ~20 GB/s.
            def load_esr(g, nsplit=2):
                e_t = esr_pool.tile([128, 4, ED], F32R, tag="esr", name=f"esr{g}")
                W = ED // nsplit
                for rr in range(4):
                    t_idx = 4 * g + rr
                    for hf in range(nsplit):
                        nc.sync.dma_start(
                            out=e_t[:, rr, W * hf:W * (hf + 1)],
                            in_=es_flat[
                                128 * t_idx:128 * (t_idx + 1),
                                W * hf:W * (hf + 1),
                            ],
                        )
                return e_t

            # Wa_bot (es half): load fp32, cast to fp8 with x64 scale into the
            # DoubleRow pair layout [128, pair, sub, U].  Triggers go on the
            # scalar+gpsimd queues to keep sync free for es/h.
            wab8 = sg.tile([128, 4, 2, U], F8)
            wab_stage = []
            for j in range(8):
                st = stg_pool.tile([128, U], F32R, tag="stg", name=f"wast{j}")
                eng = nc.scalar if j % 2 == 0 else nc.gpsimd
                for hf in range(2):
                    eng.dma_start(
                        out=st[:, 512 * hf:512 * (hf + 1)],
                        in_=wa_d.ap()[
                            U + 128 * j:U + 128 * (j + 1),
                            512 * hf:512 * (hf + 1),
                        ],
                    )
                wab_stage.append(st)

            esr = load_esr(0)

            # Wa_top tiles for qk, after es0 on sync (qk result is consumed
            # a few microseconds later than tes8)
            wat_tiles = []
            for dd in range(8):
                wat = stg_pool.tile([128, U], F32R, tag="stg", name=f"wat{dd}")
                for hf in range(2):
                    nc.sync.dma_start(
                        out=wat[:, 512 * hf:512 * (hf + 1)],
                        in_=wa_d.ap()[
                            128 * dd:128 * (dd + 1), 512 * hf:512 * (hf + 1)
                        ],
                    )
                wat_tiles.append(wat)

            in_sb = sg.tile([Bc, XD], F32, tag="scr_in_r")
            nc.sync.dma_start(out=in_sb[:], in_=inputs_d[:])

            ident = sg.tile([128, 128], F32)
            make_identity(nc, ident[:])
            identR = sg.tile([128, 128], F32R)
            nc.vector.tensor_copy(identR[:], ident[:])

            for j in range(8):
                nc.vector.tensor_scalar_mul(
                    wab8[:, j // 2, j % 2, :], wab_stage[j][:], WA_SCALE
                )

            # ---- th + small transposes (PE work with no weight deps) ----
            th = sg.tile([Bc, U], F32, tag="scr_th_z")
            nc.scalar.activation(out=th[:], in_=h_sb[:], func=AF.Tanh)

            _tp_ctr = [0]

            def transpose_to(dst, src_2d, j):
                # src_2d: [Bc, 128] -> dst[:, j, :] = src.T
                _tp_ctr[0] += 1
                pt = ps_tr.tile([128, Bc], F32, tag="tr", name=f"tp{_tp_ctr[0]}")
                nc.tensor.transpose(pt[:], src_2d, ident[:Bc, :Bc])
                nc.vector.tensor_copy(dst[:, j, :], pt[:])

            thT = sg.tile([128, 8, Bc], F32R, tag="scr_thT_ctT")
            hT = sg.tile([128, 8, Bc], F32R, tag="scr_hT_rhT")
            inT = sg.tile([128, 4, Bc], F32R, tag="scr_inT_hh")
            for j in range(8):
                transpose_to(thT, th[:, 128 * j:128 * (j + 1)], j)
            for j in range(8):
                transpose_to(hT, h_sb[:, 128 * j:128 * (j + 1)], j)
            for j in range(4):
                transpose_to(inT, in_sb[:, 128 * j:128 * (j + 1)], j)

            # qk = th @ Wa_top computed in natural layout (N=512 matmuls),
            # then transposed into qkT [u on partitions, b free]
            va_sb = sg.tile([128, 8], F32)
            nc.sync.dma_start(
                out=va_sb[:], in_=va_d.ap().rearrange("(j p) o -> p (j o)", p=128)
            )
            # dual-fp8 ldweights needs >=32 weight columns: pad with zeros,
            # only psum row 0 is meaningful
            va8 = sg.tile([128, 8, 32], F8)
            nc.vector.memset(va8[:].rearrange("p j o -> p (j o)"), 0.0)
            nc.vector.tensor_scalar_mul(
                va8[:, :, 0:1].rearrange("p j o -> p (j o)"), va_sb[:], VA_SCALE
            )
            qkT = sg.tile([128, 8, Bc], F32)
            qk_nat = sg.tile([Bc, U], F32, tag="scr_qk_ct")
            pqs = [
                ps_acc.tile([Bc, 512], F32, tag="acc", name=f"pq{h}")
                for h in range(2)
            ]
            for d in range(8):
                for half in range(2):
                    nc.tensor.matmul(
                        pqs[half][:],
                        thT[:, d, :],
                        wat_tiles[d][:, 512 * half:512 * (half + 1)],
                        start=(d == 0),
                        stop=(d == 7),
                    )
            for half in range(2):
                nc.vector.tensor_copy(
                    qk_nat[:, 512 * half:512 * (half + 1)], pqs[half][:]
                )
            for j in range(8):
                transpose_to(qkT, qk_nat[:, 128 * j:128 * (j + 1)], j)

            nxt = load_esr(1)

            bias_b = sg.tile([Bc, 3 * U], F32)
            bias_src = bias_d.ap()
            nc.sync.dma_start(
                out=bias_b[:],
                in_=bass.AP(
                    tensor=bias_src.tensor,
                    offset=bias_src.offset,
                    ap=[[0, Bc], list(bias_src.ap[0])],
                ),
            )

            # masks for block-diag A build
            masks = sg.tile([128, 4, 8], F32)
            nc.vector.memset(masks[:], 0.0)
            for rr in range(4):
                nc.vector.memset(masks[0:64, rr, 2 * rr:2 * rr + 1], 1.0)
                nc.vector.memset(masks[64:128, rr, 2 * rr + 1:2 * rr + 2], 1.0)

            ct_sb = sg.tile([Bc, ED], F32, tag="scr_qk_ct")
            gh_sb = sg.tile([Bc, 2 * U], F32, tag="scr_gh_t1")
            gx0_sb = sg.tile([Bc, 3 * U], F32)

            # GRU ct-part z/r weights, prefetched in fp8 DoubleRow pair
            # layout during the loop (x64 scale; the ct operand carries x32,
            # undone by 1/2048 in the gate STT).  hh columns stream at the
            # tail.  kb16[:, d//2, d%2, :] = kernel[512+128d:., 0:2U] * 64
            kb16 = sg.tile([128, 4, 2, 2 * U], F8)

            def kb16_block(i):
                # load order: column block (z, r) outer, d inner
                part, d = divmod(i, 8)
                st = stg_pool.tile([128, U], F32R, tag="stg", name=f"kbs{i}")
                for hf in range(2):
                    nc.scalar.dma_start(
                        out=st[:, 512 * hf:512 * (hf + 1)],
                        in_=kernel_d.ap()[
                            512 + 128 * d:512 + 128 * (d + 1),
                            U * part + 512 * hf:U * part + 512 * (hf + 1),
                        ],
                    )
                nc.vector.tensor_scalar_mul(
                    kb16[:, d // 2, d % 2, U * part:U * (part + 1)],
                    st[:], WA_SCALE,
                )

            # --- spread weight-stream blocks (emitted inside the sblk loop) ---
            def gh_block(n):
                # gh[:, n] = h @ rk[:, n-slice] + bias[n-slice]
                # issue all 8 DMAs first so the transfers run in parallel
                pg = ps_acc.tile([Bc, 512], F32, tag="acc", name=f"pg{n}")
                rkts = []
                for d in range(8):
                    rkt = wk_pool.tile(
                        [128, 512], F32R, tag="wk", name=f"rkt{n}_{d}"
                    )
                    nc.sync.dma_start(
                        out=rkt[:],
                        in_=rk_d.ap()[
                            128 * d:128 * (d + 1), 512 * n:512 * (n + 1)
                        ],
                    )
                    rkts.append(rkt)
                for d in range(8):
                    nc.tensor.matmul(
                        pg[:], hT[:, d, :], rkts[d][:],
                        start=(d == 0), stop=(d == 7),
                    )
                nc.vector.scalar_tensor_tensor(
                    out=gh_sb[:, 512 * n:512 * (n + 1)],
                    in0=pg[:],
                    scalar=1.0,
                    in1=bias_b[:, 512 * n:512 * (n + 1)],
                    op0=AluOpType.mult,
                    op1=AluOpType.add,
                )

            def gx0_block(n):
                # inputs-part of the x @ kernel gates
                pa = ps_acc.tile([Bc, 512], F32, tag="acc", name=f"gx0_{n}")
                wkts = []
                for d in range(4):
                    wkt = wk_pool.tile(
                        [128, 512], F32R, tag="wk", name=f"wk0_{n}_{d}"
                    )
                    nc.sync.dma_start(
                        out=wkt[:],
                        in_=kernel_d.ap()[
                            128 * d:128 * (d + 1), 512 * n:512 * (n + 1)
                        ],
                    )
                    wkts.append(wkt)
                for d in range(4):
                    nc.tensor.matmul(
                        pa[:], inT[:, d, :], wkts[d][:],
                        start=(d == 0), stop=(d == 3),
                    )
                nc.vector.tensor_copy(gx0_sb[:, 512 * n:512 * (n + 1)], pa[:])

            # --- pipelined alpha -> c_t pieces (ehat = unnormalized exp) ---
            def ct_head(g, ehat):
                pat = ps_ct.tile([64, 8], F32, tag="ct", name=f"pat{g}")
                nc.tensor.transpose(pat[:], ehat[:], ident[:8, :8])
                alpT2 = sm_pool.tile([128, 8], F32, tag="alT2", name=f"aT2{g}")
                nc.vector.tensor_copy(alpT2[0:64, :], pat[:])
                nc.gpsimd.dma_start(out=alpT2[64:128, :], in_=alpT2[0:64, :])
                ars = []
                for rr in range(4):
                    a_r = sm_pool.tile([128, 8], F32R, tag="A", name=f"A{g}_{rr}")
                    nc.gpsimd.tensor_mul(a_r[:], alpT2[:], masks[:, rr, :])
                    ars.append(a_r)
                return ars

            def ct_tail(g, esr_g, ars, srec, gather=True):
                # pct = sum_t ehat_t * es_t ; normalize by 1/sum(ehat) on copy
                ct_stage = sm_pool.tile(
                    [8, ED], F32, tag="ctst", name=f"cts{g}", bufs=1
                )
                for n in range(2):
                    pct = ps_ct.tile([8, 512], F32, tag="ct", name=f"pct{g}_{n}")
                    for rr in range(4):
                        nc.tensor.matmul(
                            pct[:],
                            ars[rr][:],
                            esr_g[:, rr, 512 * n:512 * (n + 1)],
                            start=(rr == 0),
                            stop=(rr == 3),
                        )
                    nc.vector.tensor_scalar_mul(
                        ct_stage[:, 512 * n:512 * (n + 1)], pct[:], srec[:]
                    )
                if gather:
                    nc.gpsimd.dma_start(
                        out=ct_sb[8 * g:8 * (g + 1), :], in_=ct_stage[:]
                    )
                return ct_stage

            # ---- attention superblock loop ----
            prev = None   # (g, esr, ehat, srec) of previous superblock
            kb_emitted = [0]
            for g in range(NSBLK):
                if g + 2 < NSBLK:
                    nxt2 = load_esr(g + 2)
                else:
                    nxt2 = None

                # tes8[p][:, q, :] = fp8(tanh(es).T) for j = 2p+q
                tes8 = []
                for p in range(4):
                    t8 = tes8_pool.tile(
                        [128, 2, 512], F8, tag="tes8", name=f"tes{g}_{p}"
                    )
                    for q in range(2):
                        j = 2 * p + q
                        pt = ps_tr.tile(
                            [128, 512], F32R, tag="tr", name=f"ptr{g}_{j}"
                        )
                        for rr in range(4):
                            nc.tensor.transpose(
                                pt[:, 128 * rr:128 * (rr + 1)],
                                esr[:, rr, 128 * j:128 * (j + 1)],
                                identR[:],
                            )
                        nc.scalar.activation(
                            out=t8[:, q, :], in_=pt[:], func=AF.Tanh
                        )
                    tes8.append(t8)

                # gt8 = fp8(tanh(Wa_bot.T @ tes / WA_SCALE + qk bcast))
                ars_prev = None
                gt8s = []
                for u in range(8):
                    if g == 0:
                        # STT(u) waits on qkT which lands late at startup:
                        # rotate over 4 psum banks (ps_e/ps_ct are idle until
                        # the first ct chain) so the u-loop runs 4 deep
                        pvpool, pvtag = [
                            (ps_v, "v"), (ps_v, "v"), (ps_e, "e"), (ps_ct, "ct")
                        ][u % 4]
                        pv = pvpool.tile(
                            [128, 512], F32, tag=pvtag, name=f"pv{g}_{u}"
                        )
                    else:
                        pv = ps_v.tile([128, 512], F32, tag="v", name=f"pv{g}_{u}")
                    for p in range(4):
                        nc.tensor.matmul(
                            pv[:],
                            wab8[:, p, :, 128 * u:128 * (u + 1)],
                            tes8[p][:],
                            start=(p == 0),
                            stop=(p == 3),
                            perf_mode=DR,
                        )
                    if u % 2 == 0:
                        g8 = gt8_pool.tile(
                            [128, 2, 512], F8, tag="gt8", name=f"gt{g}_{u // 2}"
                        )
                        gt8s.append(g8)
                    qk_slice = qkT[:, u, 8 * g:8 * g + 8]
                    qk_bc = bass.AP(
                        tensor=qk_slice.tensor,
                        offset=qk_slice.offset,
                        ap=[
                            list(qk_slice.ap[0]),
                            list(qk_slice.ap[1]),
                            [0, T],
                        ],
                    )
                    sc = stt_pool.tile(
                        [128, 512], F32, tag="stt", name=f"sc{g}_{u}"
                    )
                    nc.vector.scalar_tensor_tensor(
                        out=sc[:],
                        in0=pv[:],
                        scalar=1.0 / WA_SCALE,
                        in1=qk_bc,
                        op0=AluOpType.mult,
                        op1=AluOpType.add,
                    )
                    nc.scalar.activation(
                        out=gt8s[u // 2][:, u % 2, :], in_=sc[:], func=AF.Tanh
                    )
                    if u == 3 and prev is not None:
                        ars_prev = ct_head(prev[0], prev[2])
                    if u == 6 and prev is not None:
                        ct_tail(prev[0], prev[1], ars_prev, prev[3])

                # e*VA_SCALE = Va8.T @ gt8  (row 0 of [32, 512])
                pe = ps_e.tile([32, 512], F32, tag="e", name=f"pe{g}")
                for p in range(4):
                    nc.tensor.matmul(
                        pe[:],
                        va8[:, 2 * p:2 * p + 2, :],
                        gt8s[p][:],
                        start=(p == 0),
                        stop=(p == 3),
                        perf_mode=DR,
                    )

                # spread GRU weight streams across the attention phase
                if 4 <= g <= 7:
                    gh_block(g - 4)
                if 2 <= g <= 7:
                    gx0_block(g - 2)

                # softmax over t (|e| <~ 1.5: exp w/o max-sub is safe in fp32)
                e_sb = sm_pool.tile(
                    [1, 512], F32, tag="ctst", name=f"esb{g}", bufs=1
                )
                nc.vector.tensor_copy(e_sb[:], pe[0:1, :])
                ehat = sm_pool.tile([8, T], F32, tag="al", name=f"al{g}", bufs=2)
                nc.gpsimd.dma_start(
                    out=ehat[:],
                    in_=e_sb[0:1, :].rearrange("p (b t) -> p b t", b=8),
                )
                nc.scalar.activation(
                    out=ehat[:], in_=ehat[:], func=AF.Exp, scale=1.0 / VA_SCALE
                )
                ssum = sm_pool.tile([8, 1], F32, tag="ssum", name=f"ss{g}")
                nc.vector.reduce_sum(ssum[:], ehat[:], AX.X)
                srec = sm_pool.tile([8, 1], F32, tag="srec", name=f"sr{g}")
                nc.vector.reciprocal(srec[:], ssum[:])
                alpha = sm_pool.tile([8, T], F32, tag="alo", name=f"alo{g}", bufs=2)
                nc.gpsimd.tensor_scalar_mul(alpha[:], ehat[:], srec[:])
                nc.gpsimd.dma_start(
                    out=out_d.ap()[8 * g:8 * (g + 1), 0:T], in_=alpha[:]
                )

                # kb16 z/r tiles, emitted at the END of the superblock so the
                # scalar triggers and vector casts queue behind this block's
                # latency-critical tanh chain instead of ahead of it
                for _ in range({2: 3, 3: 3, 6: 5, 7: 5}.get(g, 0)):
                    if kb_emitted[0] < 16:
                        kb16_block(kb_emitted[0])
                        kb_emitted[0] += 1

                prev = (g, esr, ehat, srec)
                esr = nxt
                nxt = nxt2

            # fold gx0 into gh (z/r columns) and bias into gx0 (hh columns)
            # while the PE finishes g=7: one STT instead of two in the tail
            for n in range(4):
                nc.vector.scalar_tensor_tensor(
                    out=gh_sb[:, 512 * n:512 * (n + 1)],
                    in0=gh_sb[:, 512 * n:512 * (n + 1)],
                    scalar=1.0,
                    in1=gx0_sb[:, 512 * n:512 * (n + 1)],
                    op0=AluOpType.mult,
                    op1=AluOpType.add,
                )
            for n2 in range(2):
                o = 2 * U + 512 * n2
                nc.vector.scalar_tensor_tensor(
                    out=gx0_sb[:, o:o + 512],
                    in0=gx0_sb[:, o:o + 512],
                    scalar=1.0,
                    in1=bias_b[:, o:o + 512],
                    op0=AluOpType.mult,
                    op1=AluOpType.add,
                )

            # kernel-hh and rk_hh tiles: DMA into freed esr slots on the sync
            # queue (idle at the tail), 16 parallel half-triggers per tile.
            # kbh first: it is consumed earlier and gets the earlier-freed slots
            kbh_stage = []
            for half in range(2):
                ks = esr_pool.tile([128, 4, U], F32R, tag="esr", name=f"kbhs{half}")
                for rr in range(4):
                    dk = 4 * half + rr
                    for hf in range(2):
                        nc.sync.dma_start(
                            out=ks[:, rr, 512 * hf:512 * (hf + 1)],
                            in_=kernel_d.ap()[
                                512 + 128 * dk:512 + 128 * (dk + 1),
                                2 * U + 512 * hf:2 * U + 512 * (hf + 1),
                            ],
                        )
                kbh_stage.append(ks)
            rkh_tiles = []
            for half in range(2):
                rkh = esr_pool.tile([128, 4, U], F32R, tag="esr", name=f"rkh{half}")
                for rr in range(4):
                    dk = 4 * half + rr
                    for hf in range(2):
                        nc.sync.dma_start(
                            out=rkh[:, rr, 512 * hf:512 * (hf + 1)],
                            in_=rk_d.ap()[
                                128 * dk:128 * (dk + 1),
                                2 * U + 512 * hf:2 * U + 512 * (hf + 1),
                            ],
                        )
                rkh_tiles.append(rkh)

            # No-dependency warmup transposes: the PE idles during the g=7
            # softmax chain and drops to a low p-state, making the first
            # ~8µs of tail matmuls ~1.7x slower.  Keep the pipeline warm.
            for w in range(16):
                pw = ps_acc.tile([128, Bc], F32, tag="acc", name=f"warm{w}")
                nc.tensor.transpose(pw[:], ident[:Bc, :], ident[:Bc, :Bc])

            # ctT in fp8 DoubleRow pairs, x32 scale.  Rows 0..55 (g=0..6)
            # have been in ct_sb since g=6: transpose them while the g=7
            # softmax chain runs; rows 56..63 come straight from its
            # ct_stage afterwards (no gather DMA on the critical path).
            ctT = sg.tile([128, 4, 2, Bc], F8, tag="scr_thT_ctT")
            for j in range(8):
                pt = ps_tr.tile([128, 56], F32, tag="tr", name=f"tpa{j}")
                nc.tensor.transpose(
                    pt[:], ct_sb[0:56, 128 * j:128 * (j + 1)], ident[:56, :56]
                )
                nc.vector.tensor_scalar_mul(
                    ctT[:, j // 2, j % 2, 0:56], pt[:], VA_SCALE
                )

            ars_prev = ct_head(prev[0], prev[2])
            cts7 = ct_tail(prev[0], prev[1], ars_prev, prev[3], gather=False)

            for j in range(8):
                pt = ps_tr.tile([128, 8], F32, tag="tr", name=f"tpb{j}")
                nc.tensor.transpose(
                    pt[:], cts7[:, 128 * j:128 * (j + 1)], ident[:8, :8]
                )
                nc.vector.tensor_scalar_mul(
                    ctT[:, j // 2, j % 2, 56:64], pt[:], VA_SCALE
                )

            half_sb = sg.tile([Bc, 1], F32)
            nc.vector.memset(half_sb[:], 0.5)
            z_sb = sg.tile([Bc, U], F32, tag="scr_th_z")
            r_sb = sg.tile([Bc, U], F32, tag="scr_in_r")
            hh_sb = sg.tile([Bc, U], F32, tag="scr_inT_hh")

            # six gate accumulators in the (now idle) attention psum banks
            gx = [
                ps_tr.tile([Bc, 512], F32, tag="tr", name="gxa"),
                ps_tr.tile([Bc, 512], F32, tag="tr", name="gxb"),
                ps_v.tile([Bc, 512], F32, tag="v", name="gxc"),
                ps_v.tile([Bc, 512], F32, tag="v", name="gxd"),
                ps_e.tile([Bc, 512], F32, tag="e", name="gxe"),
                ps_ct.tile([Bc, 512], F32, tag="ct", name="gxf"),
            ]

            GATE_DESCALE = 1.0 / (WA_SCALE * VA_SCALE)

            def add_inplace(pa, n, src_sb, scale=1.0):
                nc.vector.scalar_tensor_tensor(
                    out=pa[:],
                    in0=pa[:],
                    scalar=scale,
                    in1=src_sb[:, 512 * n:512 * (n + 1)],
                    op0=AluOpType.mult,
                    op1=AluOpType.add,
                )

            rhT = sg.tile([128, 8, Bc], F32R, tag="scr_hT_rhT")

            def gate_block(n):
                # hard_sigmoid(gx/2048 + (gh+gx0)) = min(relu(0.2x+0.5), 1)
                dst = z_sb if n < 2 else r_sb
                o = 512 * (n % 2)
                add_inplace(gx[n], n, gh_sb, scale=GATE_DESCALE)
                nc.scalar.activation(
                    out=dst[:, o:o + 512], in_=gx[n][:],
                    func=AF.Relu, bias=half_sb[:], scale=0.2,
                )
                nc.vector.tensor_scalar_min(
                    dst[:, o:o + 512], dst[:, o:o + 512], 1.0
                )
                if n >= 2:
                    # rh = r * h for this half, then its rhT tiles scaled
                    # x2048 so the rh stream matches the fp8 hh psum scale
                    nc.vector.tensor_mul(
                        r_sb[:, o:o + 512], r_sb[:, o:o + 512],
                        h_sb[:, o:o + 512],
                    )
                    for j in range(4 * (n - 2), 4 * (n - 1)):
                        _tp_ctr[0] += 1
                        pt = ps_tr.tile(
                            [128, Bc], F32, tag="tr", name=f"tpr{j}"
                        )
                        nc.tensor.transpose(
                            pt[:], r_sb[:, 128 * j:128 * (j + 1)],
                            ident[:Bc, :Bc],
                        )
                        nc.vector.tensor_scalar_mul(
                            rhT[:, j, :], pt[:], WA_SCALE * VA_SCALE
                        )

            # cast the staged hh ct-part weight tiles to fp8 pairs (x64)
            # via the now-idle wk pool
            kbh_tiles = []
            for pr in range(4):
                kbh = wk_pool.tile([128, 2, U], F8, tag="wk", name=f"kbh{pr}")
                for q in range(2):
                    d = 2 * pr + q
                    nc.vector.tensor_scalar_mul(
                        kbh[:, q, :], kbh_stage[d // 4][:, d % 4, :], WA_SCALE
                    )
                kbh_tiles.append(kbh)

            # ct-part z/r matmuls in fp8 DoubleRow, n-outer; interleave the
            # gate vector math per completed column block
            for n in range(4):
                for p in range(4):
                    nc.tensor.matmul(
                        gx[n][:],
                        ctT[:, p, :, :],
                        kb16[:, p, :, 512 * n:512 * (n + 1)],
                        start=(p == 0),
                        stop=(p == 3),
                        perf_mode=DR,
                    )
                if n == 1:
                    gate_block(0)
                    gate_block(1)
                if n == 3:
                    gate_block(2)
                    gate_block(3)

            # hh ct-part: fp8 DoubleRow over the streamed tail tiles
            for pr in range(4):
                for n2 in range(2):
                    nc.tensor.matmul(
                        gx[4 + n2][:],
                        ctT[:, pr, :, :],
                        kbh_tiles[pr][:, :, 512 * n2:512 * (n2 + 1)],
                        start=(pr == 0),
                        stop=False,
                        perf_mode=DR,
                    )

            # hh accumulators: (r*h) @ rk_hh stream, half-outer so the first
            # half's vector chain overlaps the second half's matmuls
            t1 = sg.tile([Bc, U], F32, tag="scr_gh_t1")

            def hh_chain(n2):
                # hh = tanh(gates_hh/2048 + (gx0+bias)); h_new = hh + z*(h-hh)
                o = 512 * n2
                pa = gx[4 + n2]
                add_inplace(pa, 4 + n2, gx0_sb, scale=GATE_DESCALE)
                nc.scalar.activation(
                    out=hh_sb[:, o:o + 512], in_=pa[:], func=AF.Tanh
                )
                # chunked so the final exposure after the last matmul is short
                for ck in range(2):
                    sl = slice(o + 256 * ck, o + 256 * (ck + 1))
                    nc.vector.tensor_sub(t1[:, sl], h_sb[:, sl], hh_sb[:, sl])
                    nc.vector.tensor_mul(t1[:, sl], z_sb[:, sl], t1[:, sl])
                    nc.vector.tensor_add(t1[:, sl], hh_sb[:, sl], t1[:, sl])
                    nc.sync.dma_start(
                        out=out_d.ap()[:, T + o + 256 * ck:T + o + 256 * (ck + 1)],
                        in_=t1[:, sl],
                    )

            for n2 in range(2):
                for d in range(8):
                    nc.tensor.matmul(
                        gx[4 + n2][:],
                        rhT[:, d, :],
                        rkh_tiles[d // 4][:, d % 4, 512 * n2:512 * (n2 + 1)],
                        start=False,
                        stop=(d == 7),
                    )
                hh_chain(n2)

    return nc

_built = [None]


def kernel(**inputs):
    if _built[0] is None:
        nc = build_nc()
        fix_multi_waits(nc)
        _built[0] = nc
    nc = _built[0]

    from concourse.bass_utils import run_bass_kernel_spmd

    def f32(name):
        return np.ascontiguousarray(np.asarray(inputs[name], dtype=np.float32))

    inp = f32("inputs")
    h = f32("h")
    es = f32("encoder_states")
    ker = f32("kernel")
    rk = f32("recurrent_kernel")
    bias = f32("bias")
    wa = f32("Wa")
    va = f32("Va")

    in_maps = []
    for c in range(N_CORES):
        sl = slice(c * Bc, (c + 1) * Bc)
        in_maps.append({
            "inputs": inp[sl],
            "h": h[sl],
            "encoder_states": es[sl],
            "kernel": ker,
            "recurrent_kernel": rk,
            "bias": bias,
            "Wa": wa,
            "Va": va,
        })

    res = run_bass_kernel_spmd(nc, in_maps, list(range(N_CORES)))
    out = np.concatenate(
        [res.results[c]["out"] for c in range(N_CORES)], axis=0
    ).astype(np.float32)
    return out
